# revision 15
# baseline (speedup 1.0000x reference)
"""Trainium2 Bass kernel for DenseDilatedKnnGraph (B=4, C=192, N=M=3136, K=9).

Computes, per batch: L2-normalize x,y over channels; dist = cdist(xn, yn) +
relative_pos; output the indices of the 9 smallest distances per query row,
stacked with the center indices -> (2, B, N, 9) int32.

Sharding: query rows (N) split across 8 NeuronCores (392 rows each); y and
relative_pos rows are read per core; indices into y are global so no gather
is needed.

Per-core pipeline (per batch b, row-tile t of 98 rows):
  PE:  psum = x_tile^T @ yn              (raw x as stationary weights)
  ACT: s = sqrt(psum * (-2/||x_row||) + 2)    [d^2 = 2 - 2*cos since x2=y2=1]
  DVE: neg = (s * -1) - relpos                [negated distance]
  DVE: max8/max_index -> top-8, match_replace -> second round -> rank 9
"""

import numpy as np

import concourse.bacc as bacc
import concourse.bass as bass
import concourse.mybir as mybir
import concourse.tile as tile
from concourse.bass_utils import run_bass_kernel_spmd

B, C, N, M, K = 4, 192, 3136, 3136, 9
NCORES = 8
NB = N // NCORES  # 392 rows per core
TR = 98           # rows per compute tile
NT = NB // TR     # 4 tiles per batch
# psum column chunks: bank-aligned (512 fp32 = one 2KB bank), 3136 = 6*512+64
CHUNKS = [(i * 512, min(512, M - i * 512)) for i in range((M + 511) // 512)]
C0, C1 = 128, 64  # contraction split of C=192

F32 = mybir.dt.float32
U32 = mybir.dt.uint32
NEG_BIG = -3.0e38


def _build_kernel():
    nc = bacc.Bacc("TRN2", target_bir_lowering=False, debug=False,
                   num_devices=NCORES)
    x_ap = nc.dram_tensor("x_blk", [B, C, NB], F32, kind="ExternalInput").ap()
    y_ap = nc.dram_tensor("y_full", [B, C, M], F32, kind="ExternalInput").ap()
    rp_ap = nc.dram_tensor("relpos", [NB, M], F32, kind="ExternalInput").ap()
    out_ap = nc.dram_tensor("out_idx", [B, NB, K], U32,
                            kind="ExternalOutput").ap()

    with tile.TileContext(nc) as tc:
        _emit(tc, out_ap, x_ap, y_ap, rp_ap)
    nc.compile()
    return nc


def _emit(tc, out_ap, x_ap, y_ap, rp_ap):
    nc = tc.nc
    from contextlib import ExitStack
    with ExitStack() as ctx:
        const_p = ctx.enter_context(tc.tile_pool(name="const", bufs=1))
        rp_p = ctx.enter_context(tc.tile_pool(name="rp", bufs=1))
        x_p = ctx.enter_context(tc.tile_pool(name="x", bufs=1))
        y_p = ctx.enter_context(tc.tile_pool(name="y", bufs=1))
        yn_p = ctx.enter_context(tc.tile_pool(name="yn", bufs=1))
        big_p = ctx.enter_context(tc.tile_pool(name="big", bufs=2))
        sm_p = ctx.enter_context(tc.tile_pool(name="sm", bufs=4))
        ps_p = ctx.enter_context(tc.tile_pool(name="ps", bufs=4, space="PSUM"))
        psn_p = ctx.enter_context(tc.tile_pool(name="psn", bufs=1, space="PSUM"))

        # ---- constants ----
        ones = const_p.tile([128, 128], F32, tag="ones")
        nc.vector.memset(ones[:, :], 1.0)
        two_col = const_p.tile([TR, 1], F32, tag="two")
        nc.vector.memset(two_col[:, :], 2.0)

        # ---- persistent loads ----
        # relpos rows for this core: [98, 4, 3136]; row (t*98+p) -> [p, t, :]
        rp_sb = rp_p.tile([TR, NT, M], F32, tag="rp")
        nc.sync.dma_start(rp_sb[:, :, :],
                          rp_ap.rearrange("(t p) m -> p t m", p=TR))
        # x block, channels on partitions: [c, b, n] two contraction tiles
        x0 = x_p.tile([C0, B, NB], F32, tag="x0")
        x1 = x_p.tile([C1, B, NB], F32, tag="x1")
        x_cbn = x_ap.rearrange("b c n -> c b n")
        nc.sync.dma_start(x0[:, :, :], x_cbn[0:C0])
        nc.sync.dma_start(x1[:, :, :], x_cbn[C0:C])

        # ---- x norms: scale column -2/||x_row|| for all (b, t) ----
        sqx0 = big_p.tile([C0, B * NB], F32, tag="bigA")
        sqx1 = big_p.tile([C1, B * NB], F32, tag="bigB")
        x0f = x0[:, :, :].rearrange("c b n -> c (b n)")
        x1f = x1[:, :, :].rearrange("c b n -> c (b n)")
        nc.vector.scalar_tensor_tensor(
            sqx0[:, :], x0f, 0.0, x0f,
            op0=mybir.AluOpType.bypass, op1=mybir.AluOpType.mult)
        nc.vector.scalar_tensor_tensor(
            sqx1[:, :], x1f, 0.0, x1f,
            op0=mybir.AluOpType.bypass, op1=mybir.AluOpType.mult)
        nx2 = psn_p.tile([TR, B * NT], F32, tag="nx2")
        for b in range(B):
            for t in range(NT):
                j = b * NT + t
                lo = b * NB + t * TR
                nc.tensor.matmul(nx2[:, j:j + 1], sqx0[:, lo:lo + TR],
                                 ones[0:C0, 0:1], start=True, stop=False)
                nc.tensor.matmul(nx2[:, j:j + 1], sqx1[:, lo:lo + TR],
                                 ones[0:C1, 0:1], start=False, stop=True)
        nxr = const_p.tile([TR, B * NT], F32, tag="nxr")
        scale_col = const_p.tile([TR, B * NT], F32, tag="scale")
        for j in range(B * NT):
            nc.scalar.sqrt(nxr[:, j:j + 1], nx2[:, j:j + 1])
        nc.vector.reciprocal(nxr[:, :], nxr[:, :])
        nc.vector.tensor_scalar_mul(scale_col[:, :], nxr[:, :], -2.0)

        y_cm = y_ap  # [b, c, m]

        for b in range(B):
            # ---- normalize y (batch b) on device ----
            y0 = y_p.tile([C0, M], F32, tag="y0")
            y1 = y_p.tile([C1, M], F32, tag="y1")
            nc.sync.dma_start(y0[:, :], y_cm[b, 0:C0, :])
            nc.sync.dma_start(y1[:, :], y_cm[b, C0:C, :])
            sq0 = big_p.tile([C0, M], F32, tag="bigA")
            sq1 = big_p.tile([C1, M], F32, tag="bigB")
            nc.vector.scalar_tensor_tensor(
                sq0[:, :], y0[:, :], 0.0, y0[:, :],
                op0=mybir.AluOpType.bypass, op1=mybir.AluOpType.mult)
            nc.vector.scalar_tensor_tensor(
                sq1[:, :], y1[:, :], 0.0, y1[:, :],
                op0=mybir.AluOpType.bypass, op1=mybir.AluOpType.mult)
            ny = big_p.tile([128, M], F32, tag="bigA")
            for lo_c, sz in CHUNKS:
                cs = slice(lo_c, lo_c + sz)
                ss = ps_p.tile([128, 512], F32, tag="psmain")
                nc.tensor.matmul(ss[:, 0:sz], ones[0:C0, :], sq0[:, cs],
                                 start=True, stop=False)
                nc.tensor.matmul(ss[:, 0:sz], ones[0:C1, 0:128], sq1[:, cs],
                                 start=False, stop=True)
                nc.scalar.sqrt(ny[:, cs], ss[:, 0:sz])
            nyr = big_p.tile([128, M], F32, tag="bigB")
            nc.vector.reciprocal(nyr[:, :], ny[:, :])
            yn0 = yn_p.tile([C0, M], F32, tag="yn0")
            yn1 = yn_p.tile([C1, M], F32, tag="yn1")
            nc.vector.scalar_tensor_tensor(
                yn0[:, :], y0[:, :], 0.0, nyr[0:C0, :],
                op0=mybir.AluOpType.bypass, op1=mybir.AluOpType.mult)
            nc.vector.scalar_tensor_tensor(
                yn1[:, :], y1[:, :], 0.0, nyr[0:C1, :],
                op0=mybir.AluOpType.bypass, op1=mybir.AluOpType.mult)

            # ---- main tiles ----
            for t in range(NT):
                j = b * NT + t
                lo = b * NB + t * TR
                s_t = big_p.tile([TR, M], F32, tag="bigA")
                for lo_c, sz in CHUNKS:
                    cs = slice(lo_c, lo_c + sz)
                    pd = ps_p.tile([TR, 512], F32, tag="psmain")
                    nc.tensor.matmul(pd[:, 0:sz], x0f[:, lo:lo + TR],
                                     yn0[:, cs], start=True, stop=False)
                    nc.tensor.matmul(pd[:, 0:sz], x1f[:, lo:lo + TR],
                                     yn1[:, cs], start=False, stop=True)
                    nc.scalar.activation(s_t[:, cs], pd[:, 0:sz],
                                         mybir.ActivationFunctionType.Sqrt,
                                         bias=two_col[:, :],
                                         scale=scale_col[:, j:j + 1])
                neg = big_p.tile([TR, M], F32, tag="bigB")
                nc.vector.scalar_tensor_tensor(
                    neg[:, :], s_t[:, :], -1.0, rp_sb[:, t, :],
                    op0=mybir.AluOpType.mult, op1=mybir.AluOpType.subtract)

                v8 = sm_p.tile([TR, 8], F32, tag="v8")
                i8 = sm_p.tile([TR, 8], U32, tag="i8")
                negr = big_p.tile([TR, M], F32, tag="bigC")
                v9 = sm_p.tile([TR, 8], F32, tag="v9")
                i9 = sm_p.tile([TR, 8], U32, tag="i9")
                nc.vector.max(out=v8[:, :], in_=neg[:, :])
                nc.vector.max_index(out=i8[:, :], in_max=v8[:, :],
                                    in_values=neg[:, :])
                nc.vector.match_replace(out=negr[:, :], in_to_replace=v8[:, :],
                                        in_values=neg[:, :],
                                        imm_value=NEG_BIG)
                nc.vector.max(out=v9[:, :], in_=negr[:, :])
                nc.vector.max_index(out=i9[:, :], in_max=v9[:, :],
                                    in_values=negr[:, :])

                rows = slice(t * TR, (t + 1) * TR)
                nc.sync.dma_start(out_ap[b, rows, 0:8], i8[:, :])
                nc.sync.dma_start(out_ap[b, rows, 8:9], i9[:, 0:1])


_NC = None


def _get_nc():
    global _NC
    if _NC is None:
        _NC = _build_kernel()
    return _NC


def _run(inputs, trace=False, trace_kwargs=None):
    x = np.asarray(inputs["x"], dtype=np.float32)
    y = np.asarray(inputs["y"], dtype=np.float32)
    rp = np.asarray(inputs["relative_pos"], dtype=np.float32)
    assert x.shape == (B, C, N, 1) and y.shape == (B, C, M, 1)
    assert rp.shape == (1, N, M)

    y_full = np.ascontiguousarray(y[..., 0])
    in_maps = []
    for i in range(NCORES):
        sl = slice(i * NB, (i + 1) * NB)
        in_maps.append({
            "x_blk": np.ascontiguousarray(x[:, :, sl, 0]),
            "y_full": y_full,
            "relpos": np.ascontiguousarray(rp[0, sl, :]),
        })
    nc = _get_nc()
    kwargs = {}
    if trace:
        kwargs = dict(trace=True, trace_cores=list(range(NCORES)),
                      trace_kwargs=trace_kwargs or {})
    res = run_bass_kernel_spmd(nc, in_maps, core_ids=list(range(NCORES)),
                               **kwargs)
    nn = np.empty((B, N, K), dtype=np.int32)
    for i in range(NCORES):
        sl = slice(i * NB, (i + 1) * NB)
        nn[:, sl, :] = res.results[i]["out_idx"].view(np.int32)
    center = np.broadcast_to(np.arange(N, dtype=np.int32)[None, :, None],
                             (B, N, K))
    out = np.stack((nn, center), axis=0)
    return out, res


def kernel(**inputs):
    out, _ = _run(inputs, trace=False)
    return out


# revision 16
# speedup vs baseline: 1.2361x; 1.2361x over previous
"""Trainium2 Bass kernel for DenseDilatedKnnGraph (B=4, C=192, N=M=3136, K=9).

Computes, per batch: L2-normalize x,y over channels; dist = cdist(xn, yn) +
relative_pos; output the indices of the 9 smallest distances per query row,
stacked with the center indices -> (2, B, N, 9) int32.

Sharding: query rows (N) split across 8 NeuronCores (392 rows each); y and
relative_pos rows are read per core; indices into y are global so no gather
is needed.

Per-core pipeline (per batch b, row-tile t of 98 rows):
  PE:  psum = x_tile^T @ yn              (raw x as stationary weights)
  ACT: s = sqrt(psum * (-2/||x_row||) + 2)    [d^2 = 2 - 2*cos since x2=y2=1]
  DVE: neg = (s * -1) - relpos                [negated distance]
  DVE: max8/max_index -> top-8, match_replace -> second round -> rank 9
"""

import numpy as np

import concourse.bacc as bacc
import concourse.bass as bass
import concourse.mybir as mybir
import concourse.tile as tile
from concourse.bass_utils import run_bass_kernel_spmd

B, C, N, M, K = 4, 192, 3136, 3136, 9
NCORES = 8
NB = N // NCORES  # 392 rows per core
TR = 98           # rows per compute tile
NT = NB // TR     # 4 tiles per batch
# psum column chunks: bank-aligned (512 fp32 = one 2KB bank), 3136 = 6*512+64
CHUNKS = [(i * 512, min(512, M - i * 512)) for i in range((M + 511) // 512)]
C0, C1 = 128, 64  # contraction split of C=192

F32 = mybir.dt.float32
U32 = mybir.dt.uint32
NEG_BIG = -3.0e38


def _build_kernel():
    nc = bacc.Bacc("TRN2", target_bir_lowering=False, debug=False,
                   num_devices=NCORES)
    x_ap = nc.dram_tensor("x_blk", [B, C, NB], F32, kind="ExternalInput").ap()
    y_ap = nc.dram_tensor("y_full", [B, C, M], F32, kind="ExternalInput").ap()
    rp_ap = nc.dram_tensor("relpos", [NB, M], F32, kind="ExternalInput").ap()
    out_ap = nc.dram_tensor("out_idx", [B, NB, K], U32,
                            kind="ExternalOutput").ap()

    with tile.TileContext(nc) as tc:
        _emit(tc, out_ap, x_ap, y_ap, rp_ap)
    nc.compile()
    return nc


def _emit(tc, out_ap, x_ap, y_ap, rp_ap):
    nc = tc.nc
    from contextlib import ExitStack
    with ExitStack() as ctx:
        const_p = ctx.enter_context(tc.tile_pool(name="const", bufs=1))
        rp_p = ctx.enter_context(tc.tile_pool(name="rp", bufs=1))
        x_p = ctx.enter_context(tc.tile_pool(name="x", bufs=1))
        y_p = ctx.enter_context(tc.tile_pool(name="y", bufs=1))
        yn_p = ctx.enter_context(tc.tile_pool(name="yn", bufs=1))
        big_p = ctx.enter_context(tc.tile_pool(name="big", bufs=2))
        sm_p = ctx.enter_context(tc.tile_pool(name="sm", bufs=4))
        ps_p = ctx.enter_context(tc.tile_pool(name="ps", bufs=4, space="PSUM"))
        psn_p = ctx.enter_context(tc.tile_pool(name="psn", bufs=1, space="PSUM"))

        # ---- constants ----
        ones = const_p.tile([128, 128], F32, tag="ones")
        nc.vector.memset(ones[:, :], 1.0)
        two_col = const_p.tile([TR, 1], F32, tag="two")
        nc.vector.memset(two_col[:, :], 2.0)

        # ---- persistent loads ----
        # relpos rows for this core: [98, 4, 3136]; row (t*98+p) -> [p, t, :]
        rp_sb = rp_p.tile([TR, NT, M], F32, tag="rp")
        nc.sync.dma_start(rp_sb[:, :, :],
                          rp_ap.rearrange("(t p) m -> p t m", p=TR))
        # x block, channels on partitions: [c, b, n] two contraction tiles
        x0 = x_p.tile([C0, B, NB], F32, tag="x0")
        x1 = x_p.tile([C1, B, NB], F32, tag="x1")
        x_cbn = x_ap.rearrange("b c n -> c b n")
        nc.sync.dma_start(x0[:, :, :], x_cbn[0:C0])
        nc.sync.dma_start(x1[:, :, :], x_cbn[C0:C])

        # ---- x norms: scale column -2/||x_row|| for all (b, t) ----
        sqx0 = big_p.tile([C0, B * NB], F32, tag="bigA")
        sqx1 = big_p.tile([C1, B * NB], F32, tag="bigB")
        x0f = x0[:, :, :].rearrange("c b n -> c (b n)")
        x1f = x1[:, :, :].rearrange("c b n -> c (b n)")
        nc.vector.scalar_tensor_tensor(
            sqx0[:, :], x0f, 0.0, x0f,
            op0=mybir.AluOpType.bypass, op1=mybir.AluOpType.mult)
        nc.vector.scalar_tensor_tensor(
            sqx1[:, :], x1f, 0.0, x1f,
            op0=mybir.AluOpType.bypass, op1=mybir.AluOpType.mult)
        nx2 = psn_p.tile([TR, B * NT], F32, tag="nx2")
        for b in range(B):
            for t in range(NT):
                j = b * NT + t
                lo = b * NB + t * TR
                nc.tensor.matmul(nx2[:, j:j + 1], sqx0[:, lo:lo + TR],
                                 ones[0:C0, 0:1], start=True, stop=False)
                nc.tensor.matmul(nx2[:, j:j + 1], sqx1[:, lo:lo + TR],
                                 ones[0:C1, 0:1], start=False, stop=True)
        nxr = const_p.tile([TR, B * NT], F32, tag="nxr")
        scale_col = const_p.tile([TR, B * NT], F32, tag="scale")
        for j in range(B * NT):
            nc.scalar.sqrt(nxr[:, j:j + 1], nx2[:, j:j + 1])
        nc.vector.reciprocal(nxr[:, :], nxr[:, :])
        nc.vector.tensor_scalar_mul(scale_col[:, :], nxr[:, :], -2.0)

        y_cm = y_ap  # [b, c, m]

        for b in range(B):
            # ---- normalize y (batch b) on device ----
            y0 = y_p.tile([C0, M], F32, tag="y0")
            y1 = y_p.tile([C1, M], F32, tag="y1")
            nc.sync.dma_start(y0[:, :], y_cm[b, 0:C0, :])
            nc.sync.dma_start(y1[:, :], y_cm[b, C0:C, :])
            sq0 = big_p.tile([C0, M], F32, tag="bigA")
            sq1 = big_p.tile([C1, M], F32, tag="bigB")
            nc.vector.scalar_tensor_tensor(
                sq0[:, :], y0[:, :], 0.0, y0[:, :],
                op0=mybir.AluOpType.bypass, op1=mybir.AluOpType.mult)
            nc.vector.scalar_tensor_tensor(
                sq1[:, :], y1[:, :], 0.0, y1[:, :],
                op0=mybir.AluOpType.bypass, op1=mybir.AluOpType.mult)
            ny = big_p.tile([128, M], F32, tag="bigA")
            for lo_c, sz in CHUNKS:
                cs = slice(lo_c, lo_c + sz)
                ss = ps_p.tile([128, 512], F32, tag="psmain")
                nc.tensor.matmul(ss[:, 0:sz], ones[0:C0, :], sq0[:, cs],
                                 start=True, stop=False)
                nc.tensor.matmul(ss[:, 0:sz], ones[0:C1, 0:128], sq1[:, cs],
                                 start=False, stop=True)
                nc.scalar.sqrt(ny[:, cs], ss[:, 0:sz])
            nyr = big_p.tile([128, M], F32, tag="bigB")
            nyscr = big_p.tile([128, M], F32, tag="bigC")
            nc.vector.reciprocal_approx_accurate(nyr[:, :], ny[:, :],
                                                 nyscr[:, :])
            yn0 = yn_p.tile([C0, M], F32, tag="yn0")
            yn1 = yn_p.tile([C1, M], F32, tag="yn1")
            nc.vector.scalar_tensor_tensor(
                yn0[:, :], y0[:, :], 0.0, nyr[0:C0, :],
                op0=mybir.AluOpType.bypass, op1=mybir.AluOpType.mult)
            nc.vector.scalar_tensor_tensor(
                yn1[:, :], y1[:, :], 0.0, nyr[0:C1, :],
                op0=mybir.AluOpType.bypass, op1=mybir.AluOpType.mult)

            # ---- main tiles ----
            for t in range(NT):
                j = b * NT + t
                lo = b * NB + t * TR
                s_t = big_p.tile([TR, M], F32, tag="bigA")
                for lo_c, sz in CHUNKS:
                    cs = slice(lo_c, lo_c + sz)
                    pd = ps_p.tile([TR, 512], F32, tag="psmain")
                    nc.tensor.matmul(pd[:, 0:sz], x0f[:, lo:lo + TR],
                                     yn0[:, cs], start=True, stop=False)
                    nc.tensor.matmul(pd[:, 0:sz], x1f[:, lo:lo + TR],
                                     yn1[:, cs], start=False, stop=True)
                    nc.scalar.activation(s_t[:, cs], pd[:, 0:sz],
                                         mybir.ActivationFunctionType.Sqrt,
                                         bias=two_col[:, :],
                                         scale=scale_col[:, j:j + 1])
                neg = big_p.tile([TR, M], F32, tag="bigB")
                nc.vector.scalar_tensor_tensor(
                    neg[:, :], s_t[:, :], -1.0, rp_sb[:, t, :],
                    op0=mybir.AluOpType.mult, op1=mybir.AluOpType.subtract)

                v8 = sm_p.tile([TR, 8], F32, tag="v8")
                i8 = sm_p.tile([TR, 8], U32, tag="i8")
                negr = big_p.tile([TR, M], F32, tag="bigC")
                v9 = sm_p.tile([TR, 8], F32, tag="v9")
                i9 = sm_p.tile([TR, 8], U32, tag="i9")
                nc.vector.max(out=v8[:, :], in_=neg[:, :])
                nc.vector.max_index(out=i8[:, :], in_max=v8[:, :],
                                    in_values=neg[:, :])
                nc.vector.match_replace(out=negr[:, :], in_to_replace=v8[:, :],
                                        in_values=neg[:, :],
                                        imm_value=NEG_BIG)
                nc.vector.max(out=v9[:, :], in_=negr[:, :])
                nc.vector.max_index(out=i9[:, :], in_max=v9[:, :],
                                    in_values=negr[:, :])

                rows = slice(t * TR, (t + 1) * TR)
                nc.sync.dma_start(out_ap[b, rows, 0:8], i8[:, :])
                nc.sync.dma_start(out_ap[b, rows, 8:9], i9[:, 0:1])


_NC = None


def _get_nc():
    global _NC
    if _NC is None:
        _NC = _build_kernel()
    return _NC


def _run(inputs, trace=False, trace_kwargs=None):
    x = np.asarray(inputs["x"], dtype=np.float32)
    y = np.asarray(inputs["y"], dtype=np.float32)
    rp = np.asarray(inputs["relative_pos"], dtype=np.float32)
    assert x.shape == (B, C, N, 1) and y.shape == (B, C, M, 1)
    assert rp.shape == (1, N, M)

    y_full = np.ascontiguousarray(y[..., 0])
    in_maps = []
    for i in range(NCORES):
        sl = slice(i * NB, (i + 1) * NB)
        in_maps.append({
            "x_blk": np.ascontiguousarray(x[:, :, sl, 0]),
            "y_full": y_full,
            "relpos": np.ascontiguousarray(rp[0, sl, :]),
        })
    nc = _get_nc()
    kwargs = {}
    if trace:
        kwargs = dict(trace=True, trace_cores=list(range(NCORES)),
                      trace_kwargs=trace_kwargs or {})
    res = run_bass_kernel_spmd(nc, in_maps, core_ids=list(range(NCORES)),
                               **kwargs)
    nn = np.empty((B, N, K), dtype=np.int32)
    for i in range(NCORES):
        sl = slice(i * NB, (i + 1) * NB)
        nn[:, sl, :] = res.results[i]["out_idx"].view(np.int32)
    center = np.broadcast_to(np.arange(N, dtype=np.int32)[None, :, None],
                             (B, N, K))
    out = np.stack((nn, center), axis=0)
    return out, res


def kernel(**inputs):
    out, _ = _run(inputs, trace=False)
    return out
